# revision 23
# baseline (speedup 1.0000x reference)
"""BoundaryLoss Trainium2 kernel v2 (8 NeuronCores, data-parallel over batch).

Per core (one (21,512,512) image): ce[p] = ln(sum_c exp(x[c,p])) - x[t[p],p],
weighted by w[p] = 1 + 2*boundary[p], summed and scaled by 1/(B*H*W); the host
adds the 8 per-core partials.

Layout: channel-serial.  Pixels live in a fixed (128 partitions x 2048) map
(partition p = flat pixels [2048p, 2048p+2048) = image rows 4p..4p+3).  The
host pre-transposes x to [128, 21*2048] fp8_e4m3 so each partition's data is
one contiguous 43KB DRAM run (big descriptors -> full HBM bandwidth; fp8
halves traffic vs bf16; quantization error ~4% rms washes out in the 2.1M-
pixel mean).  Per channel c: ACT exp (fp8 in -> bf16 out), DVE mask
m=(t==c) via tensor_scalar (4x mode; all-bf16 operands), mke=m*exp via
tensor_tensor (2x mode), then identity-stationary matmuls accumulate both
exp and mke images into two [128,2048] f32 PSUM tiles (4 banks each = all 8
banks) across the 21 channels.  scalar_tensor_tensor is NEVER used for bulk
work (it has no DVE fast modes - it was the old kernel's 45us bottleneck).

Boundary map: host sends t3[p] = flat t padded +-512 at [128, 3072] bf16, so
tshm/tden/tsh are three overlapping SBUF views of ONE tensor (no broadcast
loads).  rd/rdm (DVE not_equal, 2x) -> vertical-any; OR + horizontal 3-tap +
u8 convert on GPSIMD (off the critical DVE path); borders zeroed; one EARLY
u8 AllReduce(add) of the 256KB map overlaps the main loop.  bd>0 -> w=1+2b
built on GPSIMD while the loop runs.

Tail is pipelined per PSUM bank (4x512): ln(sums)-ln(gath) (exp/ln roundtrip
keeps every DVE operand 2-byte), *w, ones-matmul partition-reduce into the
freed sums bank row 0, one ACT copy w/ accum_out + 1/N scale, DMA out.

All bulk DMA rides SWDGE (gpsimd) - 16 engines; HWDGE only for tiny consts.
"""

import sys

sys.path.insert(0, "/opt/trn_rl_repo")

import numpy as np
import ml_dtypes

import concourse.bass as bass
import concourse.bacc as bacc
import concourse.tile as tile
from concourse import mybir
from concourse import bass_utils

F32 = mybir.dt.float32
BF16 = mybir.dt.bfloat16
U8 = mybir.dt.uint8
FP8 = mybir.dt.float8e4

C = 21            # channels
H = W = 512
NPIX = H * W      # 262144 pixels per core
P = 128           # SBUF partitions
CW = NPIX // P    # 2048 pixels per partition
XW = C * CW       # 43008 bytes per partition of fp8 x
T3W = CW + 1024   # 3072: t padded with +-512 halo
NCORES = 8
NTOT = float(NCORES * NPIX)
BANK = 512        # PSUM bank width in f32

Exp = mybir.ActivationFunctionType.Exp
Ln = mybir.ActivationFunctionType.Ln
Copy = mybir.ActivationFunctionType.Copy
op = mybir.AluOpType

# x DMA split points (channels): first piece small so exp starts early
X_SPLITS = [(0, 1), (1, 4), (4, 12), (12, 21)]


def build_nc(use_cc=True):
    nc = bacc.Bacc(
        "TRN2",
        target_bir_lowering=False,
        debug=False,
        num_devices=NCORES,
        num_swdge_queues=1,
        dynamic_dma_scratch_size=16384,
    )

    x_d = nc.dram_tensor("x", [P, XW], FP8, kind="ExternalInput")
    t3_d = nc.dram_tensor("t3", [P, T3W], BF16, kind="ExternalInput")
    out_d = nc.dram_tensor("out", [1, 3], F32, kind="ExternalOutput")

    ident_d = nc.inline_tensor(np.eye(P, dtype=ml_dtypes.bfloat16), name="ident")
    ones_d = nc.inline_tensor(np.ones((P, 1), ml_dtypes.bfloat16), name="ones")
    ones32_d = nc.inline_tensor(np.ones((P, 1), np.float32), name="ones32")

    groups = [list(range(NCORES))]

    with tile.TileContext(nc) as tc:
        with (
            tc.tile_pool(name="singles", bufs=1) as singles,
            tc.tile_pool(name="expp", bufs=3) as expp,
            tc.tile_pool(name="mp", bufs=3) as mp,
            tc.tile_pool(name="tailp", bufs=2) as tailp,
            tc.tile_pool(name="psum", bufs=1, space="PSUM") as psum,
            tc.tile_pool(name="dram", bufs=1, space="DRAM") as dram,
        ):
            # tiny consts on HWDGE so the SWDGE queue starts with t3/x
            ident = singles.tile([P, P], BF16, tag="ident")
            ones = singles.tile([P, 1], BF16, tag="ones")
            ones32 = singles.tile([P, 1], F32, tag="ones32")
            nc.sync.dma_start(ident[:], ident_d[:])
            nc.sync.dma_start(ones[:], ones_d[:])
            nc.sync.dma_start(ones32[:], ones32_d[:])

            # ---- loads: t3 first (boundary + masks need it), then x ----
            t3 = singles.tile([P, T3W], BF16, tag="t3")
            nc.gpsimd.dma_start(t3[:], t3_d[:])
            x_t = singles.tile([P, XW], FP8, tag="x")
            for a, b in X_SPLITS:
                nc.gpsimd.dma_start(
                    x_t[:, a * CW : b * CW], x_d[:, a * CW : b * CW]
                )

            tshm = t3[:, 0:CW]            # flat t shifted -512 (row above)
            tden = t3[:, 512 : 512 + CW]  # flat t
            tsh = t3[:, 1024 : 1024 + CW]  # flat t shifted +512 (row below)

            # ---- boundary map ----
            # vertical any-diff on DVE (2x mode), the rest on GPSIMD so the
            # DVE queue is free for the mask ops.
            # All on DVE (cheap 2x-mode bf16 adds; masks are 0/1, `max` = OR,
            # keeping the map 0/1 for the AllReduce-max), finishing by ~10us so the AllReduce can launch
            # far ahead of when its result is needed.
            hp = tc.high_priority()
            hp.__enter__()
            rd = singles.tile([P, CW], BF16, tag="rd")
            nc.vector.tensor_tensor(rd[:], tden, tsh, op.not_equal)
            rdm = singles.tile([P, CW], BF16, tag="rdm")
            nc.vector.tensor_tensor(rdm[:], tshm, tden, op.not_equal)
            dv = singles.tile([P, CW], BF16, tag="dv")
            nc.vector.tensor_tensor(dv[:], rd[:], rdm[:], op.max)
            cat = singles.tile([P, CW], BF16, tag="cat")
            nc.vector.tensor_tensor(
                cat[:, 1 : CW - 1], dv[:, 0 : CW - 2], dv[:, 1 : CW - 1], op.max
            )
            ca = singles.tile([P, CW], BF16, tag="ca")
            nc.vector.tensor_tensor(
                ca[:, 1 : CW - 1], cat[:, 1 : CW - 1], dv[:, 2:CW], op.max
            )
            cav = ca[:].rearrange("P (r w) -> P r w", w=W)
            nc.vector.memset(cav[:, :, 0:1], 0)
            nc.vector.memset(cav[:, :, W - 1 : W], 0)
            nc.vector.memset(ca[0:1, 0:W], 0)
            # engines can't address a start partition of 127; DMA a zero row
            zrow = singles.tile([1, W], BF16, tag="zrow")
            nc.vector.memset(zrow[:], 0)
            nc.sync.dma_start(ca[P - 1 : P, 3 * W : 4 * W], zrow[:])

            # pack 2 pixels/byte (nibble counts; <=8 cores so no carry
            # between nibbles under AllReduce-add) - halves the cc payload
            # and converts bf16 -> u8 on the way.
            ca8r = ca[:].rearrange("P (n t) -> P n t", t=2)
            pkt = singles.tile([P, CW // 2], U8, tag="pkt")
            pktr = pkt[:].rearrange("P (n o) -> P n o", o=1)
            nc.vector.tensor_scalar(pktr, ca8r[:, :, 1:2], 16.0, None, op.mult)
            pk = singles.tile([P, CW // 2], U8, tag="pk")
            pkr = pk[:].rearrange("P (n o) -> P n o", o=1)
            nc.vector.tensor_tensor(pkr, ca8r[:, :, 0:1], pktr, op.add)

            # cc path rides HWDGE (sync) so it never queues behind the bulk
            # x loads on the SWDGE FIFO
            cc_in = dram.tile([P, CW // 2], U8, tag="cc_in")
            cc_out = dram.tile([P, CW // 2], U8, tag="cc_out")
            bd = singles.tile([P, CW // 2], U8, tag="bd")
            nc.sync.dma_start(cc_in[:], pk[:])
            if use_cc:
                nc.gpsimd.collective_compute(
                    "AllReduce",
                    op.add,
                    replica_groups=groups,
                    ins=[cc_in.opt()],
                    outs=[cc_out.opt()],
                )
            else:
                cc_out = cc_in
            nc.sync.dma_start(bd[:], cc_out[:])
            # u32 mask tile for the low-nibble extract (bitwise ops are
            # 32-bit only, and scalar operands must be f32, so use a full
            # tensor_tensor with a memset mask)
            U32 = mybir.dt.uint32
            lomask = singles.tile([P, CW // 8], U32, tag="lomask")
            nc.vector.memset(lomask[:], 0x0F0F0F0F)
            hp.__exit__(None, None, None)

            # ---- main loop: channels in pairs (bigger ACT/DVE ops, fewer
            # semaphore hops); 21 = 10 pairs + 1 single ----
            sums = psum.tile([P, CW], F32, tag="sums")
            gath = psum.tile([P, CW], F32, tag="gath")
            for c0 in range(0, C, 2):
                nch = min(2, C - c0)
                fw = nch * CW
                ex = expp.tile([P, 2 * CW], BF16, tag="ex")
                nc.scalar.activation(
                    ex[:, 0:fw], x_t[:, c0 * CW : (c0 + nch) * CW], Exp
                )
                m = mp.tile([P, 2 * CW], BF16, tag="m")
                for k in range(nch):
                    nc.vector.tensor_scalar(
                        m[:, k * CW : (k + 1) * CW],
                        tden,
                        float(c0 + k),
                        None,
                        op.is_equal,
                    )
                mke = mp.tile([P, 2 * CW], BF16, tag="mke")
                nc.vector.tensor_tensor(
                    mke[:, 0:fw], m[:, 0:fw], ex[:, 0:fw], op.mult
                )
                for k in range(nch):
                    c = c0 + k
                    for j in range(4):
                        s = slice(j * BANK, (j + 1) * BANK)
                        sk = slice(k * CW + j * BANK, k * CW + (j + 1) * BANK)
                        nc.tensor.matmul(
                            sums[:, s],
                            ident[:],
                            ex[:, sk],
                            start=(c == 0),
                            stop=(c == C - 1),
                            skip_group_check=True,
                        )
                    for j in range(4):
                        s = slice(j * BANK, (j + 1) * BANK)
                        sk = slice(k * CW + j * BANK, k * CW + (j + 1) * BANK)
                        nc.tensor.matmul(
                            gath[:, s],
                            ident[:],
                            mke[:, sk],
                            start=(c == 0),
                            stop=(c == C - 1),
                            skip_group_check=True,
                        )

            # ---- tail ----
            # All lns/subs first: none of these depend on the collective, so
            # they finish with the loop.  Only w4 -> wce -> fin -> copy -> out
            # sit behind bd, and the fin matmuls accumulate into one dead
            # gath bank (fresh accumulation group), not a live tile region.
            logs = singles.tile([P, CW], BF16, tag="logs")
            logg = singles.tile([P, CW], BF16, tag="logg")
            for j in range(4):
                s = slice(j * BANK, (j + 1) * BANK)
                nc.scalar.activation(logs[:, s], sums[:, s], Ln)
                nc.scalar.activation(logg[:, s], gath[:, s], Ln)
            ce = singles.tile([P, CW], BF16, tag="ce")
            for j in range(4):
                s = slice(j * BANK, (j + 1) * BANK)
                nc.vector.tensor_tensor(ce[:, s], logs[:, s], logg[:, s], op.subtract)
            # A = sum(ce) via free-dim accumulator: pre-collective
            acc = singles.tile([P, 3], F32, tag="acc")
            junka = singles.tile([P, CW], BF16, tag="junka")
            nc.vector.tensor_scalar(
                junka[:], ce[:], 1.0, None, op.mult, op.add,
                accum_out=acc[:, 0:1],
            )
            # post-collective: nibble-unpack fused into the masked accumulate.
            # odd pixels: high nibble set <=> bd > 15 (low nibble <= 8).
            # even pixels: (bd & 0x0F) > 0.
            lp = tc.high_priority(offset=-1000000)
            lp.__enter__()
            cer = ce[:].rearrange("P (n t) -> P n t", t=2)
            lo = singles.tile([P, CW // 2], U8, tag="lo")
            nc.vector.tensor_tensor(
                lo[:].bitcast(U32), bd[:].bitcast(U32), lomask[:],
                op.bitwise_and,
            )
            junkb = singles.tile([P, CW // 2], BF16, tag="junkb")
            junkbr = junkb[:].rearrange("P (n o) -> P n o", o=1)
            bdr = bd[:].rearrange("P (n o) -> P n o", o=1)
            lor = lo[:].rearrange("P (n o) -> P n o", o=1)
            nc.vector.scalar_tensor_tensor(
                junkbr, bdr, 15.0, cer[:, :, 1:2], op.is_gt, op.mult,
                accum_out=acc[:, 2:3],
            )
            nc.vector.scalar_tensor_tensor(
                junkbr, lor, 0.0, cer[:, :, 0:1], op.is_gt, op.mult,
                accum_out=acc[:, 1:2],
            )
            # partition-reduce all three sums in one tiny matmul into the
            # (dead) gath bank, scale, ship
            nc.tensor.matmul(
                gath[0:1, 0:3], ones32[:], acc[:, 0:3],
                start=True, stop=True, skip_group_check=True,
            )
            fin3 = singles.tile([1, 3], F32, tag="fin3")
            nc.scalar.activation(fin3[:], gath[0:1, 0:3], Copy, scale=1.0 / NTOT)
            nc.gpsimd.dma_start(out_d[:], fin3[:])
            lp.__exit__(None, None, None)

    nc.compile()
    return nc


_NC = None


def _get_nc():
    global _NC
    if _NC is None:
        _NC = build_nc()
    return _NC


def make_in_maps(inputs, targets):
    e4 = ml_dtypes.float8_e4m3
    in_maps = []
    for i in range(NCORES):
        x = np.asarray(inputs[i], dtype=np.float32).reshape(C, P, CW)
        x8 = np.ascontiguousarray(x.transpose(1, 0, 2)).astype(e4).reshape(P, XW)
        t = np.asarray(targets[i]).astype(np.uint8).reshape(-1)
        tp = np.zeros(NPIX + 1024, np.uint8)
        tp[512 : 512 + NPIX] = t
        t3 = np.lib.stride_tricks.as_strided(tp, (P, T3W), (CW, 1))
        t3 = np.ascontiguousarray(t3).astype(ml_dtypes.bfloat16)
        in_maps.append({"x": x8, "t3": t3})
    return in_maps


def run_device(inputs, targets, trace=False):
    nc = _get_nc()
    res = bass_utils.run_bass_kernel_spmd(
        nc,
        make_in_maps(inputs, targets),
        core_ids=list(range(NCORES)),
        trace=trace,
    )
    return res


def kernel(inputs, targets):
    res = run_device(inputs, targets, trace=False)
    # per core: out = [sum(ce), sum(b*ce) even px, sum(b*ce) odd px]/(B*H*W);
    # global mean = sum_i (A_i + 2*(Be_i + Bo_i)) over the 8 batch shards.
    return np.float32(
        sum(
            float(r["out"][0, 0])
            + 2.0 * (float(r["out"][0, 1]) + float(r["out"][0, 2]))
            for r in res.results
        )
    )


# revision 24
# speedup vs baseline: 1.0688x; 1.0688x over previous
"""BoundaryLoss Trainium2 kernel v2 (8 NeuronCores, data-parallel over batch).

Per core (one (21,512,512) image): ce[p] = ln(sum_c exp(x[c,p])) - x[t[p],p],
weighted by w[p] = 1 + 2*boundary[p], summed and scaled by 1/(B*H*W); the host
adds the 8 per-core partials.

Layout: channel-serial.  Pixels live in a fixed (128 partitions x 2048) map
(partition p = flat pixels [2048p, 2048p+2048) = image rows 4p..4p+3).  The
host pre-transposes x to [128, 21*2048] fp8_e4m3 so each partition's data is
one contiguous 43KB DRAM run (big descriptors -> full HBM bandwidth; fp8
halves traffic vs bf16; quantization error ~4% rms washes out in the 2.1M-
pixel mean).  Per channel c: ACT exp (fp8 in -> bf16 out), DVE mask
m=(t==c) via tensor_scalar (4x mode; all-bf16 operands), mke=m*exp via
tensor_tensor (2x mode), then identity-stationary matmuls accumulate both
exp and mke images into two [128,2048] f32 PSUM tiles (4 banks each = all 8
banks) across the 21 channels.  scalar_tensor_tensor is NEVER used for bulk
work (it has no DVE fast modes - it was the old kernel's 45us bottleneck).

Boundary map: host sends t3[p] = flat t padded +-512 at [128, 3072] bf16, so
tshm/tden/tsh are three overlapping SBUF views of ONE tensor (no broadcast
loads).  rd/rdm (DVE not_equal, 2x) -> vertical-any; OR + horizontal 3-tap +
u8 convert on GPSIMD (off the critical DVE path); borders zeroed; one EARLY
u8 AllReduce(add) of the 256KB map overlaps the main loop.  bd>0 -> w=1+2b
built on GPSIMD while the loop runs.

Tail is pipelined per PSUM bank (4x512): ln(sums)-ln(gath) (exp/ln roundtrip
keeps every DVE operand 2-byte), *w, ones-matmul partition-reduce into the
freed sums bank row 0, one ACT copy w/ accum_out + 1/N scale, DMA out.

All bulk DMA rides SWDGE (gpsimd) - 16 engines; HWDGE only for tiny consts.
"""

import sys

sys.path.insert(0, "/opt/trn_rl_repo")

import numpy as np
import ml_dtypes

import concourse.bass as bass
import concourse.bacc as bacc
import concourse.tile as tile
from concourse import mybir
from concourse import bass_utils

F32 = mybir.dt.float32
BF16 = mybir.dt.bfloat16
U8 = mybir.dt.uint8
FP8 = mybir.dt.float8e4

C = 21            # channels
H = W = 512
NPIX = H * W      # 262144 pixels per core
P = 128           # SBUF partitions
CW = NPIX // P    # 2048 pixels per partition
XW = C * CW       # 43008 bytes per partition of fp8 x
T3W = CW + 1024   # 3072: t padded with +-512 halo
NCORES = 8
NTOT = float(NCORES * NPIX)
BANK = 512        # PSUM bank width in f32

Exp = mybir.ActivationFunctionType.Exp
Ln = mybir.ActivationFunctionType.Ln
Copy = mybir.ActivationFunctionType.Copy
op = mybir.AluOpType

# x DMA split points (channels): first piece small so exp starts early
X_SPLITS = [(0, 1), (1, 4), (4, 12), (12, 21)]


def build_nc(use_cc=True):
    nc = bacc.Bacc(
        "TRN2",
        target_bir_lowering=False,
        debug=False,
        num_devices=NCORES,
        num_swdge_queues=1,
        dynamic_dma_scratch_size=16384,
    )

    x_d = nc.dram_tensor("x", [P, XW], FP8, kind="ExternalInput")
    t3_d = nc.dram_tensor("t3", [P, T3W], BF16, kind="ExternalInput")
    out_d = nc.dram_tensor("out", [1, 3], F32, kind="ExternalOutput")

    ident_d = nc.inline_tensor(np.eye(P, dtype=ml_dtypes.bfloat16), name="ident")
    ones_d = nc.inline_tensor(np.ones((P, 1), ml_dtypes.bfloat16), name="ones")
    ones32_d = nc.inline_tensor(np.ones((P, 1), np.float32), name="ones32")

    groups = [list(range(NCORES))]

    with tile.TileContext(nc) as tc:
        with (
            tc.tile_pool(name="singles", bufs=1) as singles,
            tc.tile_pool(name="expp", bufs=3) as expp,
            tc.tile_pool(name="mp", bufs=3) as mp,
            tc.tile_pool(name="tailp", bufs=2) as tailp,
            tc.tile_pool(name="psum", bufs=1, space="PSUM") as psum,
            tc.tile_pool(name="dram", bufs=1, space="DRAM") as dram,
        ):
            # tiny consts on HWDGE so the SWDGE queue starts with t3/x
            ident = singles.tile([P, P], BF16, tag="ident")
            ones = singles.tile([P, 1], BF16, tag="ones")
            ones32 = singles.tile([P, 1], F32, tag="ones32")
            nc.sync.dma_start(ident[:], ident_d[:])
            nc.sync.dma_start(ones[:], ones_d[:])
            nc.sync.dma_start(ones32[:], ones32_d[:])

            # ---- loads: t3 first (boundary + masks need it), then x ----
            t3 = singles.tile([P, T3W], BF16, tag="t3")
            nc.gpsimd.dma_start(t3[:], t3_d[:])
            x_t = singles.tile([P, XW], FP8, tag="x")
            for a, b in X_SPLITS:
                nc.gpsimd.dma_start(
                    x_t[:, a * CW : b * CW], x_d[:, a * CW : b * CW]
                )

            tshm = t3[:, 0:CW]            # flat t shifted -512 (row above)
            tden = t3[:, 512 : 512 + CW]  # flat t
            tsh = t3[:, 1024 : 1024 + CW]  # flat t shifted +512 (row below)

            # ---- boundary map ----
            # vertical any-diff on DVE (2x mode), the rest on GPSIMD so the
            # DVE queue is free for the mask ops.
            # All on DVE (cheap 2x-mode bf16 adds; masks are 0/1, `max` = OR,
            # keeping the map 0/1 for the AllReduce-max), finishing by ~10us so the AllReduce can launch
            # far ahead of when its result is needed.
            hp = tc.high_priority()
            hp.__enter__()
            rd = singles.tile([P, CW], BF16, tag="rd")
            nc.vector.tensor_tensor(rd[:], tden, tsh, op.not_equal)
            rdm = singles.tile([P, CW], BF16, tag="rdm")
            nc.vector.tensor_tensor(rdm[:], tshm, tden, op.not_equal)
            dv = singles.tile([P, CW], BF16, tag="dv")
            nc.vector.tensor_tensor(dv[:], rd[:], rdm[:], op.max)
            cat = singles.tile([P, CW], BF16, tag="cat")
            nc.vector.tensor_tensor(
                cat[:, 1 : CW - 1], dv[:, 0 : CW - 2], dv[:, 1 : CW - 1], op.max
            )
            ca = singles.tile([P, CW], BF16, tag="ca")
            nc.vector.tensor_tensor(
                ca[:, 1 : CW - 1], cat[:, 1 : CW - 1], dv[:, 2:CW], op.max
            )
            cav = ca[:].rearrange("P (r w) -> P r w", w=W)
            nc.vector.memset(cav[:, :, 0:1], 0)
            nc.vector.memset(cav[:, :, W - 1 : W], 0)
            nc.vector.memset(ca[0:1, 0:W], 0)
            # bottom image row (partition 127 - engines can't start there) is
            # patched with zeros in DRAM after the packed map is written
            zpk = singles.tile([1, W // 2], U8, tag="zpk")
            nc.vector.memset(zpk[:], 0)

            # pack 2 pixels/byte (nibble counts; <=8 cores so no carry
            # between nibbles under AllReduce-add) - halves the cc payload
            # and converts bf16 -> u8 on the way.
            ca8r = ca[:].rearrange("P (n t) -> P n t", t=2)
            pkt = singles.tile([P, CW // 2], U8, tag="pkt")
            pktr = pkt[:].rearrange("P (n o) -> P n o", o=1)
            nc.vector.tensor_scalar(pktr, ca8r[:, :, 1:2], 16.0, None, op.mult)
            pk = singles.tile([P, CW // 2], U8, tag="pk")
            pkr = pk[:].rearrange("P (n o) -> P n o", o=1)
            nc.vector.tensor_tensor(pkr, ca8r[:, :, 0:1], pktr, op.add)

            # cc path rides HWDGE (sync) so it never queues behind the bulk
            # x loads on the SWDGE FIFO
            cc_in = dram.tile([P, CW // 2], U8, tag="cc_in")
            cc_out = dram.tile([P, CW // 2], U8, tag="cc_out")
            bd = singles.tile([P, CW // 2], U8, tag="bd")
            nc.sync.dma_start(cc_in[:], pk[:])
            nc.sync.dma_start(
                cc_in[P - 1 : P, 3 * (W // 2) : 4 * (W // 2)], zpk[:]
            )
            if use_cc:
                nc.gpsimd.collective_compute(
                    "AllReduce",
                    op.add,
                    replica_groups=groups,
                    ins=[cc_in.opt()],
                    outs=[cc_out.opt()],
                )
            else:
                cc_out = cc_in
            nc.sync.dma_start(bd[:], cc_out[:])
            # u32 mask tile for the low-nibble extract (bitwise ops are
            # 32-bit only, and scalar operands must be f32, so use a full
            # tensor_tensor with a memset mask)
            U32 = mybir.dt.uint32
            lomask = singles.tile([P, CW // 8], U32, tag="lomask")
            nc.vector.memset(lomask[:], 0x0F0F0F0F)
            hp.__exit__(None, None, None)

            # ---- main loop: channels in pairs (bigger ACT/DVE ops, fewer
            # semaphore hops); 21 = 10 pairs + 1 single ----
            sums = psum.tile([P, CW], F32, tag="sums")
            gath = psum.tile([P, CW], F32, tag="gath")
            for c0 in range(0, C, 2):
                nch = min(2, C - c0)
                fw = nch * CW
                ex = expp.tile([P, 2 * CW], BF16, tag="ex")
                nc.scalar.activation(
                    ex[:, 0:fw], x_t[:, c0 * CW : (c0 + nch) * CW], Exp
                )
                m = mp.tile([P, 2 * CW], BF16, tag="m")
                for k in range(nch):
                    nc.vector.tensor_scalar(
                        m[:, k * CW : (k + 1) * CW],
                        tden,
                        float(c0 + k),
                        None,
                        op.is_equal,
                    )
                mke = mp.tile([P, 2 * CW], BF16, tag="mke")
                nc.vector.tensor_tensor(
                    mke[:, 0:fw], m[:, 0:fw], ex[:, 0:fw], op.mult
                )
                for k in range(nch):
                    c = c0 + k
                    for j in range(4):
                        s = slice(j * BANK, (j + 1) * BANK)
                        sk = slice(k * CW + j * BANK, k * CW + (j + 1) * BANK)
                        nc.tensor.matmul(
                            sums[:, s],
                            ident[:],
                            ex[:, sk],
                            start=(c == 0),
                            stop=(c == C - 1),
                            skip_group_check=True,
                        )
                    for j in range(4):
                        s = slice(j * BANK, (j + 1) * BANK)
                        sk = slice(k * CW + j * BANK, k * CW + (j + 1) * BANK)
                        nc.tensor.matmul(
                            gath[:, s],
                            ident[:],
                            mke[:, sk],
                            start=(c == 0),
                            stop=(c == C - 1),
                            skip_group_check=True,
                        )

            # ---- tail ----
            # All lns/subs first: none of these depend on the collective, so
            # they finish with the loop.  Only w4 -> wce -> fin -> copy -> out
            # sit behind bd, and the fin matmuls accumulate into one dead
            # gath bank (fresh accumulation group), not a live tile region.
            logs = singles.tile([P, CW], BF16, tag="logs")
            logg = singles.tile([P, CW], BF16, tag="logg")
            for j in range(4):
                s = slice(j * BANK, (j + 1) * BANK)
                nc.scalar.activation(logs[:, s], sums[:, s], Ln)
                nc.scalar.activation(logg[:, s], gath[:, s], Ln)
            ce = singles.tile([P, CW], BF16, tag="ce")
            for j in range(4):
                s = slice(j * BANK, (j + 1) * BANK)
                nc.vector.tensor_tensor(ce[:, s], logs[:, s], logg[:, s], op.subtract)
            # A = sum(ce) via free-dim accumulator: pre-collective
            acc = singles.tile([P, 3], F32, tag="acc")
            junka = singles.tile([P, CW], BF16, tag="junka")
            nc.vector.tensor_scalar(
                junka[:], ce[:], 1.0, None, op.mult, op.add,
                accum_out=acc[:, 0:1],
            )
            # post-collective: nibble-unpack fused into the masked accumulate.
            # odd pixels: high nibble set <=> bd > 15 (low nibble <= 8).
            # even pixels: (bd & 0x0F) > 0.
            lp = tc.high_priority(offset=-1000000)
            lp.__enter__()
            cer = ce[:].rearrange("P (n t) -> P n t", t=2)
            lo = singles.tile([P, CW // 2], U8, tag="lo")
            # fence: reads acc (A done), writes lo -> the bd-gated AND below
            # (WAW on lo) can't be scheduled ahead of the A accumulation
            nc.vector.tensor_scalar(
                lo[0:1, 0:4].bitcast(F32), acc[0:1, 0:1], 0.0, None, op.mult
            )
            nc.vector.tensor_tensor(
                lo[:].bitcast(U32), bd[:].bitcast(U32), lomask[:],
                op.bitwise_and,
            )
            junkb = singles.tile([P, CW // 2], BF16, tag="junkb")
            junkbr = junkb[:].rearrange("P (n o) -> P n o", o=1)
            bdr = bd[:].rearrange("P (n o) -> P n o", o=1)
            lor = lo[:].rearrange("P (n o) -> P n o", o=1)
            nc.vector.scalar_tensor_tensor(
                junkbr, bdr, 15.0, cer[:, :, 1:2], op.is_gt, op.mult,
                accum_out=acc[:, 2:3],
            )
            nc.vector.scalar_tensor_tensor(
                junkbr, lor, 0.0, cer[:, :, 0:1], op.is_gt, op.mult,
                accum_out=acc[:, 1:2],
            )
            # partition-reduce all three sums in one tiny matmul into the
            # (dead) gath bank, scale, ship
            nc.tensor.matmul(
                gath[0:1, 0:3], ones32[:], acc[:, 0:3],
                start=True, stop=True, skip_group_check=True,
            )
            fin3 = singles.tile([1, 3], F32, tag="fin3")
            nc.scalar.activation(fin3[:], gath[0:1, 0:3], Copy, scale=1.0 / NTOT)
            nc.gpsimd.dma_start(out_d[:], fin3[:])
            lp.__exit__(None, None, None)

    nc.compile()
    return nc


_NC = None


def _get_nc():
    global _NC
    if _NC is None:
        _NC = build_nc()
    return _NC


def make_in_maps(inputs, targets):
    e4 = ml_dtypes.float8_e4m3
    in_maps = []
    for i in range(NCORES):
        x = np.asarray(inputs[i], dtype=np.float32).reshape(C, P, CW)
        x8 = np.ascontiguousarray(x.transpose(1, 0, 2)).astype(e4).reshape(P, XW)
        t = np.asarray(targets[i]).astype(np.uint8).reshape(-1)
        tp = np.zeros(NPIX + 1024, np.uint8)
        tp[512 : 512 + NPIX] = t
        t3 = np.lib.stride_tricks.as_strided(tp, (P, T3W), (CW, 1))
        t3 = np.ascontiguousarray(t3).astype(ml_dtypes.bfloat16)
        in_maps.append({"x": x8, "t3": t3})
    return in_maps


def run_device(inputs, targets, trace=False):
    nc = _get_nc()
    res = bass_utils.run_bass_kernel_spmd(
        nc,
        make_in_maps(inputs, targets),
        core_ids=list(range(NCORES)),
        trace=trace,
    )
    return res


def kernel(inputs, targets):
    res = run_device(inputs, targets, trace=False)
    # per core: out = [sum(ce), sum(b*ce) even px, sum(b*ce) odd px]/(B*H*W);
    # global mean = sum_i (A_i + 2*(Be_i + Bo_i)) over the 8 batch shards.
    return np.float32(
        sum(
            float(r["out"][0, 0])
            + 2.0 * (float(r["out"][0, 1]) + float(r["out"][0, 2]))
            for r in res.results
        )
    )
